# revision 46
# baseline (speedup 1.0000x reference)
"""GAT layer kernel for Trainium2 (Bass/Tile), 8-core data-parallel over batch.

Reference (B=16, N=1024, IN_DIM=128, H=4, D=64):
    h = (x @ W).reshape(B,N,H,D)
    e_src/e_dst = einsum('bnhd,hd->bnh', h, a_src/a_dst)
    e[b,i,j,h] = leakyrelu(e_src[b,i,h] + e_dst[b,j,h], 0.2)
    alpha = softmax_j(where(adj[i,j], e, -inf));  out = alpha @ h

Kernel strategy (per core, 2 batches):
  Softmax shift-invariance: with y = s_i + d_j, lrelu(y) = 0.2 s_i + 0.2 d_j
  + 0.8 relu(y); the 0.2 s_i term is constant over j and cancels. So the
  (unnormalized) score reduces to
      PT[j,i] = max(u8_i * V_j, w_j) * m[j,i]
  with u8 = exp(0.8 e_src), V = exp(e_dst), w = exp(0.2 e_dst).

  The N^2*H elementwise score work per (b, jc) group of 128 j's:
    pass1 on DVE: tensor_scalar (mult,max fused, 4x mode) per head;
    pass2 (mask min) on DVE tensor_tensor (2x mode) for D groups or on
      GPSIMD per-head tensor_tensor for V groups.
  u8 is broadcast across partitions via one-hot K=4 matmuls + ACT exp from
  PSUM. Row-sums ride separate 1-column matmuls; normalization is a
  reciprocal + scale (ACT for b0, DVE broadcast multiply for b1).
  All heavy matmuls use bf16 or fp32r (1 PE cycle/row vs 4 for fp32).
"""

import os
import sys
from contextlib import ExitStack

import numpy as np
import ml_dtypes

for _p in ("/opt/trn_rl_repo", "/root/.axon_site/_ro/trn_rl_repo"):
    if os.path.isdir(_p) and _p not in sys.path:
        sys.path.insert(0, _p)

import concourse.bass as bass
import concourse.mybir as mybir
import concourse.tile as tile

F32 = mybir.dt.float32
F32R = mybir.dt.float32r
BF16 = mybir.dt.bfloat16
AF = mybir.ActivationFunctionType
ALU = mybir.AluOpType
NPBF = ml_dtypes.bfloat16

B, N, IN_DIM, H, D = 16, 1024, 128, 4, 64
HD = H * D            # 256
NCORES = 8
BL = B // NCORES      # 2 batches per core
NTC = N // 128        # 8 chunks of 128

# score-group engine assignment: (b, jc) -> 'D' | 'V'
#   D: pass2 mask-min on DVE (one tensor_tensor over all 4 heads)
#   V: pass2 mask-min on GPSIMD (per-head tensor_tensor, eff 0.6)
MODE = {}
for _b in range(BL):
    for _jc in range(NTC):
        MODE[(_b, _jc)] = "D"
V_GROUPS = [(0, 0), (0, 1), (1, 0), (1, 1)]
H_GROUPS = []   # DVE masks heads 0-1, GPSIMD heads 2-3
QT_BUFS = 3
QWM_BUFS = 4
OSB_BUFS = 3
VQT_BUFS = 2   # per-batch tag
for _g in V_GROUPS:
    MODE[_g] = "V"


def _split_excess_waits(nc, max_waits=1):
    """Walrus codegen rejects compute instructions carrying more than one
    sync wait. Move the extras onto engine-matched NoOps inserted
    immediately before the instruction."""
    def _steal_nop(engine):
        engine.nop()
        for fn in nc.m.functions:
            for blk in fn.blocks:
                il = blk.instructions
                if il and type(il[-1]).__name__ == "InstNoOp":
                    nop = il[-1]
                    blk.instructions = il[:-1]
                    return nop
        raise RuntimeError("could not locate appended nop")

    for fn in nc.m.functions:
        for blk in fn.blocks:
            il = list(blk.instructions)
            out = []
            changed = False
            for inst in il:
                si = inst.sync_info
                if (type(inst).__name__ != "InstNoOp" and si is not None
                        and len(si.on_wait) > max_waits):
                    waits = list(si.on_wait)
                    for w in waits[max_waits:]:
                        nop = _steal_nop(nc.engines[inst.engine])
                        nop.sync_info = mybir.SyncInfo(on_wait=[w], on_update=[])
                        out.append(nop)
                    inst.sync_info = mybir.SyncInfo(
                        on_wait=waits[:max_waits], on_update=list(si.on_update))
                    changed = True
                out.append(inst)
            if changed:
                blk.instructions = out
    return nc


def build_gat_program():
    nc = bass.Bass("TRN2", target_bir_lowering=False, debug=False)
    xT_d = nc.dram_tensor("xT", (BL, IN_DIM, N), F32R, kind="ExternalInput").ap()
    W_d = nc.dram_tensor("W", (IN_DIM, HD), F32R, kind="ExternalInput").ap()
    WAcat_d = nc.dram_tensor("WAcat", (IN_DIM, 36), F32R, kind="ExternalInput").ap()
    maskT_d = nc.dram_tensor("maskT", (N, N), BF16, kind="ExternalInput").ap()
    onehot_d = nc.dram_tensor("onehot", (4, 4 * 128), F32R, kind="ExternalInput").ap()
    out_d = nc.dram_tensor("out", (BL, N, HD), BF16, kind="ExternalOutput").ap()

    with tile.TileContext(nc) as tc:
        with ExitStack() as ctx:
            _gat_body(ctx, tc, out_d, xT_d, W_d, WAcat_d, maskT_d, onehot_d)
    _split_excess_waits(nc)
    return nc


def _gat_body(ctx, tc, out_d, xT_d, W_d, WAcat_d, maskT_d, onehot_d):
    nc = tc.nc

    consts = ctx.enter_context(tc.tile_pool(name="consts", bufs=1))
    persist = ctx.enter_context(tc.tile_pool(name="persist", bufs=1))
    qt_pool = ctx.enter_context(tc.tile_pool(name="qt", bufs=QT_BUFS))
    qwm_pool = ctx.enter_context(tc.tile_pool(name="qwm", bufs=QWM_BUFS))
    osb_pool = ctx.enter_context(tc.tile_pool(name="osb", bufs=OSB_BUFS))
    rcl_pool = ctx.enter_context(tc.tile_pool(name="rcl", bufs=3))
    ps_z = ctx.enter_context(tc.tile_pool(name="ps_z", bufs=2, space="PSUM"))
    ps_p1 = ctx.enter_context(tc.tile_pool(name="ps_p1", bufs=1, space="PSUM"))
    ps_acc = ctx.enter_context(tc.tile_pool(name="ps_acc", bufs=1, space="PSUM"))

    # ---- constants / inputs resident in SBUF ----
    # DMA transfers are serviced serially; order so b0's inputs land first.
    WAcat_sb = consts.tile([128, 36], F32R)
    nc.sync.dma_start(out=WAcat_sb, in_=WAcat_d)
    xT_sb = consts.tile([128, BL, N], F32R)
    nc.sync.dma_start(out=xT_sb[:, 0, 0:512], in_=xT_d[0][:, 0:512])
    nc.sync.dma_start(out=xT_sb[:, 0, 512:], in_=xT_d[0][:, 512:])
    onehot_sb = consts.tile([4, 4 * 128], F32R)
    nc.sync.dma_start(out=onehot_sb, in_=onehot_d)
    W_sb = consts.tile([128, HD], F32R)
    nc.sync.dma_start(out=W_sb, in_=W_d)
    ones_col = consts.tile([128, 1], BF16)
    nc.vector.memset(ones_col, 1.0)
    maskT_sb = consts.tile([128, NTC, N], BF16)
    maskT_src = maskT_d.rearrange("(jc p) i -> p jc i", p=128)

    def load_mask(jc):
        nc.sync.dma_start(out=maskT_sb[:, jc, :], in_=maskT_src[:, jc, :])

    for _jc in range(4):
        load_mask(_jc)

    # ---- persistent per-batch intermediates ----
    haug_sb = persist.tile([128, BL, NTC, HD], BF16)   # [j-in-chunk, b, jc, h*64+d]
    srow_sb = persist.tile([4, BL, N], F32R)           # raw e_src rows
    Vcol_sb = persist.tile([128, BL, NTC, H], F32)     # exp(e_dst) cols
    wcol_sb = persist.tile([128, BL, NTC, H], F32)     # exp(0.2 e_dst) cols
    U8bc = persist.tile([128, BL, H, N], BF16)         # u8 broadcast over parts

    # ---- phase 1: E = x @ WAcat (rows + cols), haug = x @ W ----
    for b in range(BL):
        # E rows [a=src4+dst4, t] via two 512-col halves (z-pool slots)
        for half in range(2):
            e8 = ps_z.tile([128, 512], F32, tag="z", name=f"e8_{b}_{half}")
            nc.tensor.matmul(e8[0:36, :], lhsT=WAcat_sb,
                             rhs=xT_sb[:, b, half * 512:(half + 1) * 512],
                             start=True, stop=True)
            sl = slice(half * 512, (half + 1) * 512)
            if b == 0 and half == 0:
                # halves copied on different engines -> parallel latency on
                # the critical path to the first score op
                nc.vector.tensor_copy(srow_sb[0:4, b, sl], e8[0:4, :])
            else:
                nc.scalar.activation(srow_sb[0:4, b, sl], e8[0:4, :], AF.Copy,
                                     bias=0.0, scale=1.0)
        def emit_ebc(h):
            # broadcast e_src row h across partitions via one-hot K=4 matmul,
            # then exp(0.8 x) straight from PSUM into the bf16 U8bc tile
            for half in range(2):
                sl = slice(half * 512, (half + 1) * 512)
                ebc = ps_z.tile([128, 512], F32, tag="z", name=f"ebc_{b}_{h}_{half}")
                nc.tensor.matmul(ebc, lhsT=onehot_sb[:, h * 128:(h + 1) * 128],
                                 rhs=srow_sb[0:4, b, sl], start=True, stop=True)
                nc.scalar.activation(U8bc[:, b, h, sl], ebc, AF.Exp,
                                     bias=0.0, scale=0.8)

        # head-0 broadcast first: it gates the first pass1 tensor_scalar
        emit_ebc(0)
        # E cols [t, a] per 128-chunk; exp into V / w columns
        ecol_slot = ps_z.tile([128, 512], F32, tag="z", name=f"ecol_{b}")
        ecol = ecol_slot[:, 0:NTC * 36]
        for tcc in range(NTC):
            nc.tensor.matmul(ecol[:, tcc * 36:(tcc + 1) * 36],
                             lhsT=xT_sb[:, b, tcc * 128:(tcc + 1) * 128],
                             rhs=WAcat_sb, start=True, stop=True)
        dstv = ecol.rearrange("p (t a) -> p t a", t=NTC)[:, :, 32:36]
        nc.scalar.activation(Vcol_sb[:, b], dstv, AF.Exp, bias=0.0, scale=1.0)
        nc.scalar.activation(wcol_sb[:, b], dstv, AF.Exp, bias=0.0, scale=0.2)
        if b == 0:
            load_mask(4)
            load_mask(5)
            nc.sync.dma_start(out=xT_sb[:, 1, :], in_=xT_d[1])
            load_mask(6)
            load_mask(7)
        # haug copies interleaved so haug[jc] lands before group jc needs it
        for h in range(H):
            if h > 0:
                emit_ebc(h)
            for tcc in (2 * h, 2 * h + 1):
                hp = ps_p1.tile([128, HD], F32, tag="haug")
                nc.tensor.matmul(hp, lhsT=xT_sb[:, b, tcc * 128:(tcc + 1) * 128],
                                 rhs=W_sb, start=True, stop=True)
                nc.scalar.activation(haug_sb[:, b, tcc, :], hp, AF.Copy,
                                     bias=0.0, scale=1.0)

    # ---- phase 2: scores + alpha @ h ----
    # V-group (GPSIMD-masked) qwm tiles are allocated up front and their
    # matmuls issued at the END of each batch so slow GPSIMD production
    # neither blocks the qwm ring nor stalls PE behind unready groups.
    vqwm = {}

    def produce_v(b, jc):
        vjcs_b = [j for j in range(NTC) if MODE[(b, j)] == "V"]
        qt = qt_pool.tile([128, H, N], BF16, tag=f"vqt{b}", bufs=VQT_BUFS,
                          name=f"vqt_{b}_{jc}")
        for h in range(H):
            nc.vector.tensor_scalar(
                out=qt[:, h, :], in0=U8bc[:, b, h, :],
                scalar1=Vcol_sb[:, b, jc, h:h + 1],
                scalar2=wcol_sb[:, b, jc, h:h + 1],
                op0=ALU.mult, op1=ALU.max)
        qwm = qwm_pool.tile([128, H, N], BF16, tag=f"vqwm{b}", bufs=len(vjcs_b),
                            name=f"vqwm_{b}_{jc}")
        for h in range(H):
            nc.gpsimd.tensor_tensor(
                out=qwm[:, h, :], in0=qt[:, h, :],
                in1=maskT_sb[:, jc, :], op=ALU.mult)
        vqwm[(b, jc)] = qwm

    for _jc in [jc for jc in range(NTC) if MODE[(0, jc)] == "V"]:
        produce_v(0, _jc)
    for b in range(BL):
        # 4 oacc banks hold the 32 (ic,h) 64-col chains; rs holds row-sums
        obank = [ps_acc.tile([128, 512], F32, tag=f"oacc{k}", name=f"oacc{k}_{b}")
                 for k in range(4)]
        rs = ps_acc.tile([128, 32], F32, tag="rs")
        vjcs = [jc for jc in range(NTC) if MODE[(b, jc)] == "V"]
        djcs = [jc for jc in range(NTC) if MODE[(b, jc)] != "V"]

        def issue_matmuls(jc, qwm, pos, last):
            # start=True zeroes a whole 2KB psum bank: only the first chain
            # in each bank starts the group, only the last one stops it. In
            # the final group the row-sum matmuls go first so the reciprocal
            # can start while the data chains finish.
            passes = [("rs",), ("data",)] if last else [("data", "rs")]
            for kinds in passes:
                for h in range(H):
                    for ic in range(NTC):
                        c = ic * 4 + h
                        lhsT = qwm[:, h, ic * 128:(ic + 1) * 128]
                        if "data" in kinds:
                            nc.tensor.matmul(
                                obank[c // 8][:, (c % 8) * 64:(c % 8 + 1) * 64],
                                lhsT=lhsT,
                                rhs=haug_sb[:, b, jc, h * 64:(h + 1) * 64],
                                start=(pos == 0 and c % 8 == 0),
                                stop=(last and c % 8 == 7))
                        if "rs" in kinds:
                            nc.tensor.matmul(rs[:, c:c + 1], lhsT=lhsT,
                                             rhs=ones_col,
                                             start=(pos == 0 and c == 0),
                                             stop=(last and c == 31))

        # V-groups issue mid-order: their qwm is ready early (GPSIMD), so
        # PE passes through them without stalling, and the kernel tail is
        # gated only by the final D group
        order = djcs[:4] + vjcs + djcs[4:]
        for pos, jc in enumerate(order):
            # b1's V-group score work is emitted early (during b0's D-group
            # stretch) so GPSIMD stays fed across the batch boundary
            if b == 0 and pos == 2:
                for _jc in [j for j in range(NTC) if MODE[(1, j)] == "V"]:
                    produce_v(1, _jc)
            if MODE[(b, jc)] == "V":
                qwm = vqwm.pop((b, jc))
            else:
                qt = qt_pool.tile([128, H, N], BF16, tag="qt")
                for h in range(H):
                    nc.vector.tensor_scalar(
                        out=qt[:, h, :], in0=U8bc[:, b, h, :],
                        scalar1=Vcol_sb[:, b, jc, h:h + 1],
                        scalar2=wcol_sb[:, b, jc, h:h + 1],
                        op0=ALU.mult, op1=ALU.max)
                qwm = qwm_pool.tile([128, H, N], BF16, tag="qwm")
                if (b, jc) in H_GROUPS:
                    nc.vector.tensor_tensor(
                        out=qwm[:, 0:2, :], in0=qt[:, 0:2, :],
                        in1=maskT_sb[:, jc, :].unsqueeze(1)
                            .broadcast_to((128, 2, N)),
                        op=ALU.mult)
                    for h in (2, 3):
                        nc.gpsimd.tensor_tensor(
                            out=qwm[:, h, :], in0=qt[:, h, :],
                            in1=maskT_sb[:, jc, :], op=ALU.mult)
                elif pos == NTC - 1:
                    # per-head masks on the final group: its first matmuls
                    # start ~1.6us earlier, shortening the batch tail
                    for h in range(H):
                        nc.vector.tensor_tensor(
                            out=qwm[:, h, :], in0=qt[:, h, :],
                            in1=maskT_sb[:, jc, :], op=ALU.mult)
                else:
                    nc.vector.tensor_tensor(
                        out=qwm, in0=qt,
                        in1=maskT_sb[:, jc, :].unsqueeze(1)
                            .broadcast_to((128, H, N)),
                        op=ALU.mult)
            issue_matmuls(jc, qwm, pos, pos == NTC - 1)
        rcl = rcl_pool.tile([128, 32], F32, tag="rcl")
        nc.vector.reciprocal(rcl, rs)
        osb = osb_pool.tile([128, NTC, HD], BF16, tag="osb")
        half_out = NTC // 2
        if b == 0:
            # b0 norm on ACT (overlaps b1 scores); b1 on the DVE tail
            for ic in range(NTC):
                oslice = obank[ic // 2][:, (ic % 2) * 256:(ic % 2 + 1) * 256]
                for h in range(H):
                    nc.scalar.activation(
                        osb[:, ic, h * 64:(h + 1) * 64],
                        oslice[:, h * 64:(h + 1) * 64], AF.Copy,
                        bias=0.0, scale=rcl[:, ic * 4 + h:ic * 4 + h + 1])
                if ic == half_out - 1:
                    nc.sync.dma_start(
                        out=out_d[b, 0:half_out * 128].rearrange(
                            "(ic p) d -> p ic d", p=128),
                        in_=osb[:, 0:half_out, :])
        else:
            # whole-bank normalize (2 ics per op) shortens the serial tail
            for bk in range(4):
                nc.vector.tensor_tensor(
                    out=osb[:, 2 * bk:2 * bk + 2, :]
                        .rearrange("p ic (h d) -> p ic h d", h=H),
                    in0=obank[bk].rearrange("p (ic h d) -> p ic h d", ic=2, h=H),
                    in1=rcl[:, 8 * bk:8 * (bk + 1)]
                        .rearrange("p (ic h) -> p ic h", ic=2).unsqueeze(3)
                        .broadcast_to((128, 2, 4, D)), op=ALU.mult)
                if bk == 1:
                    nc.sync.dma_start(
                        out=out_d[b, 0:half_out * 128].rearrange(
                            "(ic p) d -> p ic d", p=128),
                        in_=osb[:, 0:half_out, :])
        nc.sync.dma_start(
            out=out_d[b, half_out * 128:].rearrange("(ic p) d -> p ic d", p=128),
            in_=osb[:, half_out:, :])


def prep_inputs(x, adj, W, a_src, a_dst):
    """Host-side prep: shard x over cores, build combined weight layouts."""
    x = np.asarray(x, np.float32)
    adj = np.asarray(adj)
    W = np.asarray(W, np.float32)
    a_src = np.asarray(a_src, np.float32)
    a_dst = np.asarray(a_dst, np.float32)

    maskT = np.ascontiguousarray(adj.T).astype(NPBF)
    Acat = np.zeros((HD, 36), np.float32)
    for h in range(H):
        Acat[h * D:(h + 1) * D, h] = a_src[h]
        Acat[h * D:(h + 1) * D, 32 + h] = a_dst[h]
    WAcat = np.ascontiguousarray(W @ Acat)  # (IN_DIM, 36): src at 0-3, dst at 32-35

    onehot = np.zeros((4, 4 * 128), np.float32)
    for h in range(H):
        onehot[h, h * 128:(h + 1) * 128] = 1.0

    in_maps = []
    for c in range(NCORES):
        xT = np.ascontiguousarray(x[c * BL:(c + 1) * BL].transpose(0, 2, 1))
        in_maps.append({"xT": xT, "W": W, "WAcat": WAcat, "maskT": maskT,
                        "onehot": onehot})
    return in_maps


_PROGRAM_CACHE = {}


def _get_program():
    if "nc" not in _PROGRAM_CACHE:
        _PROGRAM_CACHE["nc"] = build_gat_program()
    return _PROGRAM_CACHE["nc"]


def run_on_hw(inputs, trace=False):
    from concourse.bass_utils import run_bass_kernel_spmd
    nc = _get_program()
    in_maps = prep_inputs(**inputs)
    res = run_bass_kernel_spmd(nc, in_maps, list(range(NCORES)), trace=trace)
    out = np.concatenate(
        [np.asarray(res.results[c]["out"]).astype(np.float32)
         for c in range(NCORES)], axis=0)
    return out, res


def kernel(**inputs) -> np.ndarray:
    out, _ = run_on_hw(inputs, trace=False)
    return out


# revision 56
# speedup vs baseline: 1.0000x; 1.0000x over previous
"""GAT layer kernel for Trainium2 (Bass/Tile), 8-core data-parallel over batch.

Reference (B=16, N=1024, IN_DIM=128, H=4, D=64):
    h = (x @ W).reshape(B,N,H,D)
    e_src/e_dst = einsum('bnhd,hd->bnh', h, a_src/a_dst)
    e[b,i,j,h] = leakyrelu(e_src[b,i,h] + e_dst[b,j,h], 0.2)
    alpha = softmax_j(where(adj[i,j], e, -inf));  out = alpha @ h

Kernel strategy (per core, 2 batches):
  Softmax shift-invariance: with y = s_i + d_j, lrelu(y) = 0.2 s_i + 0.2 d_j
  + 0.8 relu(y); the 0.2 s_i term is constant over j and cancels. So the
  (unnormalized) score reduces to
      PT[j,i] = max(u8_i * V_j, w_j) * m[j,i]
  with u8 = exp(0.8 e_src), V = exp(e_dst), w = exp(0.2 e_dst).

  The N^2*H elementwise score work per (b, jc) group of 128 j's:
    pass1 on DVE: tensor_scalar (mult,max fused, 4x mode) per head;
    pass2 (mask multiply) on DVE tensor_tensor (2x mode) for D groups or
      on GPSIMD per-head tensor_tensor for V groups, issued so PE never
      stalls behind the slower GPSIMD groups.
  u8 is broadcast across partitions via one-hot K=4 matmuls + ACT exp from
  PSUM. Row-sums ride separate 1-column matmuls; normalization is a
  reciprocal + scale (ACT for b0, DVE broadcast multiply for b1).
  All heavy matmuls use bf16 or fp32r (1 PE cycle/row vs 4 for fp32).
"""

import os
import sys
from contextlib import ExitStack

import numpy as np
import ml_dtypes

for _p in ("/opt/trn_rl_repo", "/root/.axon_site/_ro/trn_rl_repo"):
    if os.path.isdir(_p) and _p not in sys.path:
        sys.path.insert(0, _p)

import concourse.bass as bass
import concourse.mybir as mybir
import concourse.tile as tile

F32 = mybir.dt.float32
F32R = mybir.dt.float32r
BF16 = mybir.dt.bfloat16
AF = mybir.ActivationFunctionType
ALU = mybir.AluOpType
NPBF = ml_dtypes.bfloat16

B, N, IN_DIM, H, D = 16, 1024, 128, 4, 64
HD = H * D            # 256
NCORES = 8
BL = B // NCORES      # 2 batches per core
NTC = N // 128        # 8 chunks of 128

# score-group engine assignment: (b, jc) -> 'D' | 'V'
#   D: pass2 mask-min on DVE (one tensor_tensor over all 4 heads)
#   V: pass2 mask-min on GPSIMD (per-head tensor_tensor, eff 0.6)
MODE = {}
for _b in range(BL):
    for _jc in range(NTC):
        MODE[(_b, _jc)] = "D"
V_GROUPS = [(0, 0), (0, 1), (1, 0), (1, 1)]
H_GROUPS = []   # DVE masks heads 0-1, GPSIMD heads 2-3
QT_BUFS = 3
QWM_BUFS = 4
OSB_BUFS = 3
VQT_BUFS = 2   # per-batch tag
for _g in V_GROUPS:
    MODE[_g] = "V"


def _split_excess_waits(nc, max_waits=1):
    """Walrus codegen rejects compute instructions carrying more than one
    sync wait. Move the extras onto engine-matched NoOps inserted
    immediately before the instruction."""
    def _steal_nop(engine):
        engine.nop()
        for fn in nc.m.functions:
            for blk in fn.blocks:
                il = blk.instructions
                if il and type(il[-1]).__name__ == "InstNoOp":
                    nop = il[-1]
                    blk.instructions = il[:-1]
                    return nop
        raise RuntimeError("could not locate appended nop")

    for fn in nc.m.functions:
        for blk in fn.blocks:
            il = list(blk.instructions)
            out = []
            changed = False
            for inst in il:
                si = inst.sync_info
                if (type(inst).__name__ != "InstNoOp" and si is not None
                        and len(si.on_wait) > max_waits):
                    waits = list(si.on_wait)
                    for w in waits[max_waits:]:
                        nop = _steal_nop(nc.engines[inst.engine])
                        nop.sync_info = mybir.SyncInfo(on_wait=[w], on_update=[])
                        out.append(nop)
                    inst.sync_info = mybir.SyncInfo(
                        on_wait=waits[:max_waits], on_update=list(si.on_update))
                    changed = True
                out.append(inst)
            if changed:
                blk.instructions = out
    return nc


def build_gat_program():
    nc = bass.Bass("TRN2", target_bir_lowering=False, debug=False)
    xT_d = nc.dram_tensor("xT", (BL, IN_DIM, N), F32R, kind="ExternalInput").ap()
    W_d = nc.dram_tensor("W", (IN_DIM, HD), F32R, kind="ExternalInput").ap()
    WAcat_d = nc.dram_tensor("WAcat", (IN_DIM, 36), F32R, kind="ExternalInput").ap()
    maskT_d = nc.dram_tensor("maskT", (N, N), BF16, kind="ExternalInput").ap()
    onehot_d = nc.dram_tensor("onehot", (4, 4 * 128), F32R, kind="ExternalInput").ap()
    out_d = nc.dram_tensor("out", (BL, N, HD), BF16, kind="ExternalOutput").ap()

    with tile.TileContext(nc) as tc:
        with ExitStack() as ctx:
            _gat_body(ctx, tc, out_d, xT_d, W_d, WAcat_d, maskT_d, onehot_d)
    _split_excess_waits(nc)
    return nc


def _gat_body(ctx, tc, out_d, xT_d, W_d, WAcat_d, maskT_d, onehot_d):
    nc = tc.nc

    consts = ctx.enter_context(tc.tile_pool(name="consts", bufs=1))
    persist = ctx.enter_context(tc.tile_pool(name="persist", bufs=1))
    qt_pool = ctx.enter_context(tc.tile_pool(name="qt", bufs=QT_BUFS))
    qwm_pool = ctx.enter_context(tc.tile_pool(name="qwm", bufs=QWM_BUFS))
    osb_pool = ctx.enter_context(tc.tile_pool(name="osb", bufs=OSB_BUFS))
    rcl_pool = ctx.enter_context(tc.tile_pool(name="rcl", bufs=3))
    ps_z = ctx.enter_context(tc.tile_pool(name="ps_z", bufs=2, space="PSUM"))
    ps_p1 = ctx.enter_context(tc.tile_pool(name="ps_p1", bufs=1, space="PSUM"))
    ps_acc = ctx.enter_context(tc.tile_pool(name="ps_acc", bufs=1, space="PSUM"))

    # ---- constants / inputs resident in SBUF ----
    # DMA transfers are serviced serially; order so b0's inputs land first.
    WAcat_sb = consts.tile([128, 36], F32R)
    nc.sync.dma_start(out=WAcat_sb, in_=WAcat_d)
    xT_sb = consts.tile([128, BL, N], F32R)
    nc.sync.dma_start(out=xT_sb[:, 0, 0:512], in_=xT_d[0][:, 0:512])
    nc.sync.dma_start(out=xT_sb[:, 0, 512:], in_=xT_d[0][:, 512:])
    onehot_sb = consts.tile([4, 4 * 128], F32R)
    nc.sync.dma_start(out=onehot_sb, in_=onehot_d)
    W_sb = consts.tile([128, HD], F32R)
    nc.sync.dma_start(out=W_sb, in_=W_d)
    ones_col = consts.tile([128, 1], BF16)
    nc.vector.memset(ones_col, 1.0)
    maskT_sb = consts.tile([128, NTC, N], BF16)
    maskT_src = maskT_d.rearrange("(jc p) i -> p jc i", p=128)

    def load_mask(jc):
        nc.sync.dma_start(out=maskT_sb[:, jc, :], in_=maskT_src[:, jc, :])

    for _jc in range(4):
        load_mask(_jc)

    # ---- persistent per-batch intermediates ----
    haug_sb = persist.tile([128, BL, NTC, HD], BF16)   # [j-in-chunk, b, jc, h*64+d]
    srow_sb = persist.tile([4, BL, N], F32R)           # raw e_src rows
    Vcol_sb = persist.tile([128, BL, NTC, H], F32)     # exp(e_dst) cols
    wcol_sb = persist.tile([128, BL, NTC, H], F32)     # exp(0.2 e_dst) cols
    U8bc = persist.tile([128, BL, H, N], BF16)         # u8 broadcast over parts

    # ---- phase 1: E = x @ WAcat (rows + cols), haug = x @ W ----
    for b in range(BL):
        # E rows [a=src4+dst4, t] via two 512-col halves (z-pool slots)
        for half in range(2):
            e8 = ps_z.tile([128, 512], F32, tag="z", name=f"e8_{b}_{half}")
            nc.tensor.matmul(e8[0:36, :], lhsT=WAcat_sb,
                             rhs=xT_sb[:, b, half * 512:(half + 1) * 512],
                             start=True, stop=True)
            sl = slice(half * 512, (half + 1) * 512)
            if b == 0:
                # DVE is idle this early; keep the ACT queue free for the
                # exps that gate the first score op
                nc.vector.tensor_copy(srow_sb[0:4, b, sl], e8[0:4, :])
            else:
                nc.scalar.activation(srow_sb[0:4, b, sl], e8[0:4, :], AF.Copy,
                                     bias=0.0, scale=1.0)
        def emit_ebc(h):
            # broadcast e_src row h across partitions via one-hot K=4 matmul,
            # then exp(0.8 x) straight from PSUM into the bf16 U8bc tile
            for half in range(2):
                sl = slice(half * 512, (half + 1) * 512)
                ebc = ps_z.tile([128, 512], F32, tag="z", name=f"ebc_{b}_{h}_{half}")
                nc.tensor.matmul(ebc, lhsT=onehot_sb[:, h * 128:(h + 1) * 128],
                                 rhs=srow_sb[0:4, b, sl], start=True, stop=True)
                nc.scalar.activation(U8bc[:, b, h, sl], ebc, AF.Exp,
                                     bias=0.0, scale=0.8)

        # head-0 broadcast first: it gates the first pass1 tensor_scalar
        emit_ebc(0)
        # E cols [t, a] per 128-chunk; exp into V / w columns
        ecol_slot = ps_z.tile([128, 512], F32, tag="z", name=f"ecol_{b}")
        ecol = ecol_slot[:, 0:NTC * 36]
        for tcc in range(NTC):
            nc.tensor.matmul(ecol[:, tcc * 36:(tcc + 1) * 36],
                             lhsT=xT_sb[:, b, tcc * 128:(tcc + 1) * 128],
                             rhs=WAcat_sb, start=True, stop=True)
        dstv = ecol.rearrange("p (t a) -> p t a", t=NTC)[:, :, 32:36]
        nc.scalar.activation(Vcol_sb[:, b], dstv, AF.Exp, bias=0.0, scale=1.0)
        nc.scalar.activation(wcol_sb[:, b], dstv, AF.Exp, bias=0.0, scale=0.2)
        if b == 0:
            load_mask(4)
            load_mask(5)
            nc.sync.dma_start(out=xT_sb[:, 1, :], in_=xT_d[1])
            load_mask(6)
            load_mask(7)
        # haug copies interleaved so haug[jc] lands before group jc needs it
        for h in range(H):
            if h > 0:
                emit_ebc(h)
            for tcc in (2 * h, 2 * h + 1):
                hp = ps_p1.tile([128, HD], F32, tag="haug")
                nc.tensor.matmul(hp, lhsT=xT_sb[:, b, tcc * 128:(tcc + 1) * 128],
                                 rhs=W_sb, start=True, stop=True)
                nc.scalar.activation(haug_sb[:, b, tcc, :], hp, AF.Copy,
                                     bias=0.0, scale=1.0)

    # ---- phase 2: scores + alpha @ h ----
    # V-group (GPSIMD-masked) qwm tiles are allocated up front and their
    # matmuls issued at the END of each batch so slow GPSIMD production
    # neither blocks the qwm ring nor stalls PE behind unready groups.
    vqwm = {}

    def produce_v(b, jc):
        vjcs_b = [j for j in range(NTC) if MODE[(b, j)] == "V"]
        qt = qt_pool.tile([128, H, N], BF16, tag=f"vqt{b}", bufs=VQT_BUFS,
                          name=f"vqt_{b}_{jc}")
        for h in range(H):
            nc.vector.tensor_scalar(
                out=qt[:, h, :], in0=U8bc[:, b, h, :],
                scalar1=Vcol_sb[:, b, jc, h:h + 1],
                scalar2=wcol_sb[:, b, jc, h:h + 1],
                op0=ALU.mult, op1=ALU.max)
        qwm = qwm_pool.tile([128, H, N], BF16, tag=f"vqwm{b}", bufs=len(vjcs_b),
                            name=f"vqwm_{b}_{jc}")
        for h in range(H):
            nc.gpsimd.tensor_tensor(
                out=qwm[:, h, :], in0=qt[:, h, :],
                in1=maskT_sb[:, jc, :], op=ALU.mult)
        vqwm[(b, jc)] = qwm

    for _jc in [jc for jc in range(NTC) if MODE[(0, jc)] == "V"]:
        produce_v(0, _jc)
    for b in range(BL):
        # 4 oacc banks hold the 32 (ic,h) 64-col chains; rs holds row-sums
        obank = [ps_acc.tile([128, 512], F32, tag=f"oacc{k}", name=f"oacc{k}_{b}")
                 for k in range(4)]
        rs = ps_acc.tile([128, 32], F32, tag="rs")
        vjcs = [jc for jc in range(NTC) if MODE[(b, jc)] == "V"]
        djcs = [jc for jc in range(NTC) if MODE[(b, jc)] != "V"]

        def issue_matmuls(jc, qwm, pos, last):
            # start=True zeroes a whole 2KB psum bank: only the first chain
            # in each bank starts the group, only the last one stops it. In
            # the final group the row-sum matmuls go first so the reciprocal
            # can start while the data chains finish.
            passes = [("rs",), ("data",)] if last else [("data", "rs")]
            for kinds in passes:
                for h in range(H):
                    for ic in range(NTC):
                        c = ic * 4 + h
                        lhsT = qwm[:, h, ic * 128:(ic + 1) * 128]
                        if "data" in kinds:
                            nc.tensor.matmul(
                                obank[c // 8][:, (c % 8) * 64:(c % 8 + 1) * 64],
                                lhsT=lhsT,
                                rhs=haug_sb[:, b, jc, h * 64:(h + 1) * 64],
                                start=(pos == 0 and c % 8 == 0),
                                stop=(last and c % 8 == 7))
                        if "rs" in kinds:
                            nc.tensor.matmul(rs[:, c:c + 1], lhsT=lhsT,
                                             rhs=ones_col,
                                             start=(pos == 0 and c == 0),
                                             stop=(last and c == 31))

        # V-groups issue mid-order: their qwm is ready early (GPSIMD), so
        # PE passes through them without stalling, and the kernel tail is
        # gated only by the final D group
        order = djcs[:4] + vjcs + djcs[4:]
        for pos, jc in enumerate(order):
            # b1's V-group score work is emitted early (during b0's D-group
            # stretch) so GPSIMD stays fed across the batch boundary
            if b == 0 and pos == 2:
                for _jc in [j for j in range(NTC) if MODE[(1, j)] == "V"]:
                    produce_v(1, _jc)
            if MODE[(b, jc)] == "V":
                qwm = vqwm.pop((b, jc))
            else:
                qt = qt_pool.tile([128, H, N], BF16, tag="qt")
                for h in range(H):
                    nc.vector.tensor_scalar(
                        out=qt[:, h, :], in0=U8bc[:, b, h, :],
                        scalar1=Vcol_sb[:, b, jc, h:h + 1],
                        scalar2=wcol_sb[:, b, jc, h:h + 1],
                        op0=ALU.mult, op1=ALU.max)
                qwm = qwm_pool.tile([128, H, N], BF16, tag="qwm")
                if (b, jc) in H_GROUPS:
                    nc.vector.tensor_tensor(
                        out=qwm[:, 0:2, :], in0=qt[:, 0:2, :],
                        in1=maskT_sb[:, jc, :].unsqueeze(1)
                            .broadcast_to((128, 2, N)),
                        op=ALU.mult)
                    for h in (2, 3):
                        nc.gpsimd.tensor_tensor(
                            out=qwm[:, h, :], in0=qt[:, h, :],
                            in1=maskT_sb[:, jc, :], op=ALU.mult)
                elif pos == NTC - 1:
                    # per-head masks on the final group: its first matmuls
                    # start ~1.6us earlier, shortening the batch tail
                    for h in range(H):
                        nc.vector.tensor_tensor(
                            out=qwm[:, h, :], in0=qt[:, h, :],
                            in1=maskT_sb[:, jc, :], op=ALU.mult)
                else:
                    nc.vector.tensor_tensor(
                        out=qwm, in0=qt,
                        in1=maskT_sb[:, jc, :].unsqueeze(1)
                            .broadcast_to((128, H, N)),
                        op=ALU.mult)
            issue_matmuls(jc, qwm, pos, pos == NTC - 1)
        rcl = rcl_pool.tile([128, 32], F32, tag="rcl")
        nc.vector.reciprocal(rcl, rs)
        osb = osb_pool.tile([128, NTC, HD], BF16, tag="osb")
        half_out = NTC // 2
        if b == 0:
            # b0 norm on ACT (overlaps b1 scores); b1 on the DVE tail
            for ic in range(NTC):
                oslice = obank[ic // 2][:, (ic % 2) * 256:(ic % 2 + 1) * 256]
                for h in range(H):
                    nc.scalar.activation(
                        osb[:, ic, h * 64:(h + 1) * 64],
                        oslice[:, h * 64:(h + 1) * 64], AF.Copy,
                        bias=0.0, scale=rcl[:, ic * 4 + h:ic * 4 + h + 1])
                if ic == half_out - 1:
                    nc.sync.dma_start(
                        out=out_d[b, 0:half_out * 128].rearrange(
                            "(ic p) d -> p ic d", p=128),
                        in_=osb[:, 0:half_out, :])
        else:
            # whole-bank normalize (2 ics per op) shortens the serial tail
            for bk in range(4):
                nc.vector.tensor_tensor(
                    out=osb[:, 2 * bk:2 * bk + 2, :]
                        .rearrange("p ic (h d) -> p ic h d", h=H),
                    in0=obank[bk].rearrange("p (ic h d) -> p ic h d", ic=2, h=H),
                    in1=rcl[:, 8 * bk:8 * (bk + 1)]
                        .rearrange("p (ic h) -> p ic h", ic=2).unsqueeze(3)
                        .broadcast_to((128, 2, 4, D)), op=ALU.mult)
                if bk == 1:
                    nc.sync.dma_start(
                        out=out_d[b, 0:half_out * 128].rearrange(
                            "(ic p) d -> p ic d", p=128),
                        in_=osb[:, 0:half_out, :])
        nc.sync.dma_start(
            out=out_d[b, half_out * 128:].rearrange("(ic p) d -> p ic d", p=128),
            in_=osb[:, half_out:, :])


def prep_inputs(x, adj, W, a_src, a_dst):
    """Host-side prep: shard x over cores, build combined weight layouts."""
    x = np.asarray(x, np.float32)
    adj = np.asarray(adj)
    W = np.asarray(W, np.float32)
    a_src = np.asarray(a_src, np.float32)
    a_dst = np.asarray(a_dst, np.float32)

    maskT = np.ascontiguousarray(adj.T).astype(NPBF)
    Acat = np.zeros((HD, 36), np.float32)
    for h in range(H):
        Acat[h * D:(h + 1) * D, h] = a_src[h]
        Acat[h * D:(h + 1) * D, 32 + h] = a_dst[h]
    WAcat = np.ascontiguousarray(W @ Acat)  # (IN_DIM, 36): src at 0-3, dst at 32-35

    onehot = np.zeros((4, 4 * 128), np.float32)
    for h in range(H):
        onehot[h, h * 128:(h + 1) * 128] = 1.0

    in_maps = []
    for c in range(NCORES):
        xT = np.ascontiguousarray(x[c * BL:(c + 1) * BL].transpose(0, 2, 1))
        in_maps.append({"xT": xT, "W": W, "WAcat": WAcat, "maskT": maskT,
                        "onehot": onehot})
    return in_maps


_PROGRAM_CACHE = {}


def _get_program():
    if "nc" not in _PROGRAM_CACHE:
        _PROGRAM_CACHE["nc"] = build_gat_program()
    return _PROGRAM_CACHE["nc"]


def run_on_hw(inputs, trace=False):
    from concourse.bass_utils import run_bass_kernel_spmd
    nc = _get_program()
    in_maps = prep_inputs(**inputs)
    res = run_bass_kernel_spmd(nc, in_maps, list(range(NCORES)), trace=trace)
    out = np.concatenate(
        [np.asarray(res.results[c]["out"]).astype(np.float32)
         for c in range(NCORES)], axis=0)
    return out, res


def kernel(**inputs) -> np.ndarray:
    out, _ = run_on_hw(inputs, trace=False)
    return out
